# revision 24
# baseline (speedup 1.0000x reference)
"""GCN layer (gather + segment-sum + degree-normalize + linear) on 8 Trainium2 cores.

Strategy
--------
Destination-node sharding: core k owns dest rows [k*D, (k+1)*D), D = n_nodes/8.
The host groups each core's edges by 128-dest windows (dest-sorted); the
on-device segment-sum is done per 128-edge chunk with a PE matmul
(lhsT = gathered source features G [128 edge, 128 feat] bf16, rhs = selection
matrix S [128 edge, 128 dest] with S[e, j] = (col_rel[e] == j)), accumulating
aggT[feat, dest] in PSUM per window. S is built on DVE in 32-chunk batches
(one tensor_tensor is_equal of broadcast crel vs a constant iota row per
batch — batching amortizes the ~151-cycle DVE instruction overhead to
~140ns/chunk). 1/max(deg,1) is precomputed on the host (a pure function of
edge_index, like the gather indices) and applied per window as a
per-partition activation scale fused with the PSUM->SBUF copy on the Scalar
engine after the linear matmul. No PE transpose is needed: aggT in PSUM is
copied to SBUF (Scalar) and used directly as lhsT of the linear matmul
(out[j, f'] = sum_f aggT[f, j] * wt[f, f']). Bias rides as a replicated
[128, out_f] DVE add. No scatter-add, no collectives; each core writes its
own output slice and the host concatenates.

x is gathered in bf16 (256B/row). Measured on HW, dma_gather throughput is
descriptor-count-bound (~2.1-3 ns/desc sustained at 4 SWDGE queues,
regardless of 256B/512B payload or call size), so one 256B bf16 descriptor
per edge is the floor; bf16 keeps SDMA engine busy-time at half of the 512B
hi+lo split-f32 scheme's and gives rel err ~2.5e-3 (max-abs vs output
scale), well inside the 2e-2 gate. The gather stream is the critical path:
index tiles load first (the first call waits only on them), G blocks are
triple-buffered per stream so the serial Pool dispatcher never head-of-line
blocks on a consumer dependency, and DVE/PE/Scalar work (S-builds, window
accumulation, epilogues) all ride under the ~230us descriptor stream.

dma_gather facts (measured): idx arrays are int16, wrapped [16, N/16] and
replicated into all eight 16-partition groups; single_packet=False is required
for calls over 1024 indices. int16 limits a gather call's index range to
32768 rows, so edges are split into lo/hi source streams gathered from base
x[0] / x[32768].
"""
import sys
import os
sys.path.insert(0, "/opt/trn_rl_repo")

import numpy as np

P = 128
GATHER_SPLIT = 32768       # max rows addressable by a signed-int16 gather index
DEFAULT_BLK_CHUNKS = 32    # gather block size in 128-edge chunks
SBATCH = 32                # S-matrix build batch, in chunks (amortizes the
                           # ~151-cycle DVE instruction overhead)
N_CORES = 8


def _ceil_div(a, b):
    return -(-a // b)


def _wrap_idx(ix):
    """[N] int16 -> [128, N/16], idx i at [i%16, i//16], replicated into the
    eight 16-partition groups (the tx/rx Q7 cpus of every SWDGE queue each
    read their own group)."""
    n = len(ix)
    assert n % 16 == 0
    w = np.zeros((P, n // 16), np.int16)
    blk = ix.reshape(-1, 16).T
    for g in range(8):
        w[16 * g:16 * (g + 1), :] = blk
    return w


class Plan:
    """Host-side sharding: per-core per-stream edge arrays with a chunk
    structure (windows x chunk counts) identical across cores, so a single
    SPMD program serves all cores."""

    def __init__(self, row, col, n_nodes, n_cores=N_CORES,
                 blk_chunks=DEFAULT_BLK_CHUNKS, gather_split=GATHER_SPLIT):
        assert n_nodes % n_cores == 0
        self.n_cores = n_cores
        self.n_nodes = n_nodes
        self.d_core = n_nodes // n_cores
        self.n_win = _ceil_div(self.d_core, P)
        self.blk_chunks = blk_chunks
        self.gather_split = gather_split

        order = np.argsort(col, kind="stable")
        rs = row[order]
        cs = col[order]
        bounds = np.searchsorted(cs, np.arange(n_cores + 1) * self.d_core)

        # in-degree (clamped to 1) per node, laid out per core as
        # [P, n_win] f32 reciprocal: recip[j, w] = 1/deg of dest w*128+j
        deg = np.bincount(cs, minlength=n_nodes).astype(np.float32)
        deg = np.maximum(deg, 1.0)
        recip = (1.0 / deg)
        pad = self.n_win * P - self.d_core
        self.core_recip = []
        for k in range(n_cores):
            r = recip[k * self.d_core:(k + 1) * self.d_core]
            r = np.concatenate([r, np.zeros(pad, np.float32)])
            self.core_recip.append(
                np.ascontiguousarray(r.reshape(self.n_win, P).T))

        W = self.n_win
        per_core = []  # [k][stream] = (rows, local_cols, per-window counts)
        cnt = {"lo": np.zeros(W, np.int64), "hi": np.zeros(W, np.int64)}
        for k in range(n_cores):
            a, b = bounds[k], bounds[k + 1]
            r_k = rs[a:b]
            lc_k = cs[a:b] - k * self.d_core
            lo = r_k < gather_split
            streams = {}
            for sname, mask in (("lo", lo), ("hi", ~lo)):
                r_s = r_k[mask]
                lc_s = lc_k[mask]
                counts = np.bincount(lc_s >> 7, minlength=W)
                streams[sname] = (r_s, lc_s, counts)
                cnt[sname] = np.maximum(cnt[sname], -(-counts // P))
            per_core.append(streams)
        cnt["lo"] = np.maximum(cnt["lo"], 1)  # every window gets >=1 chunk
        self.cnt = cnt
        self.off = {s: np.concatenate([[0], np.cumsum(cnt[s])]) for s in cnt}
        self.Csum = {s: int(self.off[s][-1]) for s in cnt}
        self.NB = {s: _ceil_div(self.Csum[s], blk_chunks) for s in cnt}
        self.Npad = {s: self.NB[s] * blk_chunks * P for s in cnt}

        self.core_arrays = []
        for k in range(n_cores):
            arrs = {}
            for sname in ("lo", "hi"):
                r_s, lc_s, counts = per_core[k][sname]
                off = self.off[sname]
                base = 0 if sname == "lo" else gather_split
                gidx = np.zeros(self.Npad[sname], np.int16)
                crel = np.full(self.Csum[sname] * P, -1, np.int8)
                if len(r_s):
                    starts = np.concatenate([[0], np.cumsum(counts)])
                    adj = off[:-1] * P - starts[:-1]
                    dst = np.arange(len(r_s)) + adj[lc_s >> 7]
                    gidx[dst] = (r_s - base).astype(np.int16)
                    crel[dst] = (lc_s & 127).astype(np.int8)
                arrs[f"gidx_{sname}"] = _wrap_idx(gidx)
                # [P edge-slot, Csum chunk] f32 (tensor_scalar is_equal
                # requires a float32 scalar operand)
                arrs[f"crel_{sname}"] = np.ascontiguousarray(
                    crel.reshape(self.Csum[sname], P).T.astype(np.float32))
            self.core_arrays.append(arrs)

    @property
    def total_chunks(self):
        return self.Csum["lo"] + self.Csum["hi"]


def _patch_swdge_lane_by_queue():
    """Pin each dma_gather's DMASW semaphore lane to its SWDGE queue number.

    Tile assigns DMASW lanes round-robin in scheduled order, which breaks when
    instructions on different queues (whose completions are only FIFO within a
    queue) share a lane. Two lanes per queue keep per-lane completion in-order
    and let a call's desc-gen overlap the previous same-queue call's DMA
    completion.
    """
    import concourse.tile_sem_assignment as tsa
    from concourse import mybir
    if getattr(tsa.TileClockTick, "_lane_by_queue_patch", False):
        return
    orig = tsa.TileClockTick._assign_tick

    def patched(self, inst):
        if isinstance(inst, mybir.InstDMAGatherAnt):
            if not hasattr(self, "_q_lane_ctr"):
                self._q_lane_ctr = {}
            q = inst.queue_num
            n = self._q_lane_ctr.get(q, 0)
            self._q_lane_ctr[q] = n + 1
            saved = self.next_sw_dma_idx
            self.next_sw_dma_idx = q * 2 + (n % 2)
            try:
                return orig(self, inst)
            finally:
                self.next_sw_dma_idx = saved
        return orig(self, inst)

    tsa.TileClockTick._assign_tick = patched
    tsa.TileClockTick._lane_by_queue_patch = True


def build_program(plan, in_f, out_f):
    """Emit the SPMD Bass program (shared by all cores)."""
    from concourse import bacc, mybir
    import concourse.tile as tile
    from contextlib import ExitStack

    _patch_swdge_lane_by_queue()

    f32 = mybir.dt.float32
    i16 = mybir.dt.int16
    bf16 = mybir.dt.bfloat16

    W = plan.n_win
    BLK = plan.blk_chunks

    nc = bacc.Bacc("TRN2", target_bir_lowering=False, debug=False,
                   num_devices=plan.n_cores, num_swdge_queues=4)

    x_d = nc.dram_tensor("xb", [plan.n_nodes, in_f], bf16,
                         kind="ExternalInput")
    wt_d = nc.dram_tensor("wt", [in_f, out_f], bf16, kind="ExternalInput")
    bias_d = nc.dram_tensor("bias", [P, out_f], f32, kind="ExternalInput")
    iota_d = nc.dram_tensor("iota", [P, P], f32, kind="ExternalInput")
    recip_d = nc.dram_tensor("recip", [P, W], f32, kind="ExternalInput")
    gidx_d, crel_d = {}, {}
    for s in ("lo", "hi"):
        if plan.Csum[s] == 0:
            continue
        gidx_d[s] = nc.dram_tensor(f"gidx_{s}", [P, plan.Npad[s] // 16], i16,
                                   kind="ExternalInput")
        crel_d[s] = nc.dram_tensor(f"crel_{s}", [P, plan.Csum[s]], f32,
                                   kind="ExternalInput")
    out_d = nc.dram_tensor("out", [W * P, out_f], f32, kind="ExternalOutput")

    x_base = {"lo": x_d[:], "hi": x_d[plan.gather_split:, :]}

    with tile.TileContext(nc) as tc, ExitStack() as ctx:
        cpool = ctx.enter_context(tc.tile_pool(name="const", bufs=1))
        gpool = {s: ctx.enter_context(tc.tile_pool(name=f"g_{s}", bufs=3))
                 for s in ("lo", "hi")}
        spool = {s: ctx.enter_context(tc.tile_pool(name=f"s_{s}", bufs=3))
                 for s in ("lo", "hi")}
        epool = ctx.enter_context(tc.tile_pool(name="epi", bufs=3))
        apool = ctx.enter_context(tc.tile_pool(name="psum_a", bufs=4,
                                               space="PSUM"))
        hpool = ctx.enter_context(tc.tile_pool(name="psum_h", bufs=2,
                                               space="PSUM"))

        # ---- constants ----
        # gidx first: the first gather call (the critical-path wall) waits
        # only on its index tile, not on the whole constant load train.
        gidx_t, crel_t = {}, {}
        for s in ("lo", "hi"):
            if plan.Csum[s] == 0:
                continue
            git = cpool.tile([P, plan.Npad[s] // 16], i16, name=f"gidx{s}")
            nc.sync.dma_start(out=git[:], in_=gidx_d[s][:])
            gidx_t[s] = git
        for s in ("lo", "hi"):
            if plan.Csum[s] == 0:
                continue
            cri = cpool.tile([P, plan.Csum[s]], f32, name=f"crel{s}")
            nc.sync.dma_start(out=cri[:], in_=crel_d[s][:])
            crel_t[s] = cri
        iota_t = cpool.tile([P, P], f32)
        nc.sync.dma_start(out=iota_t[:], in_=iota_d[:])
        wt_t = cpool.tile([in_f, out_f], bf16)
        nc.sync.dma_start(out=wt_t[:], in_=wt_d[:])
        bias_t = cpool.tile([P, out_f], f32)
        nc.sync.dma_start(out=bias_t[:], in_=bias_d[:])
        recip_t = cpool.tile([P, W], f32)
        nc.sync.dma_start(out=recip_t[:], in_=recip_d[:])

        # ---- lazily-emitted gather blocks ----
        # Queue assignment is static per (stream, block parity) so each pool
        # tag's DMA semaphore lane stays on one SWDGE queue.
        g_tiles = {}
        qctr = [0]
        HB = BLK // 2  # half-block chunks; one gather call per half, own queue

        def get_g(s, b):
            if (s, b) not in g_tiles:
                gt = gpool[s].tile([P, BLK * in_f], bf16, name=f"G{s}{b}",
                                   tag=f"G{s}{b % 2}")
                for h in range(2):
                    c0 = b * BLK + h * HB
                    nch = min(HB, max(plan.Csum[s] - c0, 0))
                    if nch == 0:
                        continue
                    nc.gpsimd.dma_gather(
                        gt[:, h * HB * in_f:(h * HB + nch) * in_f]
                        .rearrange("p (c e) -> p c e", e=in_f),
                        x_base[s],
                        gidx_t[s][:, c0 * P // 16:(c0 + nch) * P // 16],
                        nch * P,
                        nch * P,
                        in_f,
                        single_packet=False,
                        queue_num=qctr[0] % 4,
                    )
                    qctr[0] += 1
                g_tiles[(s, b)] = gt
            return g_tiles[(s, b)]

        # ---- lazily-emitted batched S builds (one DVE tensor_tensor
        # is_equal per SBATCH chunks; the ~151-cycle instruction overhead
        # amortizes to ~138ns/chunk) ----
        s_tiles = {}

        def get_s(s, sb):
            if (s, sb) not in s_tiles:
                st = spool[s].tile([P, SBATCH * P], bf16, name=f"S{s}{sb}",
                                   tag=f"S{s}")
                nb = min(SBATCH, plan.Csum[s] - sb * SBATCH)
                in0 = crel_t[s][:, sb * SBATCH:sb * SBATCH + nb] \
                    .to_broadcast([P, nb, P])
                in1 = iota_t[:][:, None, :].to_broadcast([P, nb, P])
                outv = st[:].rearrange("p (b j) -> p b j", j=P)[:, :nb, :]
                nc.vector.tensor_tensor(out=outv, in0=in0, in1=in1,
                                        op=mybir.AluOpType.is_equal)
                s_tiles[(s, sb)] = st
            return s_tiles[(s, sb)]

        # ---- pre-issue every gather call, interleaved by stream progress,
        # so the Pool engine always has ready calls on all 4 queues ----
        order = sorted(
            [(s, b) for s in ("lo", "hi") for b in range(plan.NB[s])],
            key=lambda sb: (sb[1] + 0.5) / plan.NB[sb[0]])
        for s, b in order:
            get_g(s, b)

        # ---- main window loop ----
        for w in range(W):
            chunks = []
            for s in ("lo", "hi"):
                chunks += [(s, c) for c in
                           range(plan.off[s][w], plan.off[s][w + 1])]
            psum_aggT = apool.tile([P, in_f], f32, tag="aggT",
                                   name=f"aggT{w}")
            n = len(chunks)
            for i, (s, c) in enumerate(chunks):
                b, slot = divmod(c, BLK)
                sb, ssub = divmod(c, SBATCH)
                gt = get_g(s, b)
                st = get_s(s, sb)
                nc.tensor.matmul(
                    out=psum_aggT[:],
                    lhsT=gt[:, slot * in_f:(slot + 1) * in_f],
                    rhs=st[:, ssub * P:(ssub + 1) * P],
                    start=(i == 0), stop=(i == n - 1))

            hT_t = epool.tile([P, P], bf16, tag="hT", name=f"hT{w}")
            nc.scalar.activation(out=hT_t[:], in_=psum_aggT[:],
                                 func=mybir.ActivationFunctionType.Copy)
            out_p = hpool.tile([P, out_f], f32, tag="outp", name=f"outp{w}")
            nc.tensor.matmul(out=out_p[:], lhsT=hT_t[:], rhs=wt_t[:],
                             start=True, stop=True)
            out_s = epool.tile([P, out_f], f32, tag="outs", name=f"outs{w}")
            nc.scalar.activation(out=out_s[:], in_=out_p[:],
                                 func=mybir.ActivationFunctionType.Copy,
                                 scale=recip_t[:, w:w + 1])
            out_t = epool.tile([P, out_f], f32, tag="outt", name=f"outt{w}")
            nc.vector.tensor_tensor(out=out_t[:], in0=out_s[:], in1=bias_t[:],
                                    op=mybir.AluOpType.add)
            nc.sync.dma_start(out=out_d[w * P:(w + 1) * P, :], in_=out_t[:])

    nc.compile()
    return nc


def make_in_maps(plan, x, W, b):
    in_f = x.shape[1]
    out_f = W.shape[0]
    import ml_dtypes
    xb = np.ascontiguousarray(x, dtype=np.float32).astype(ml_dtypes.bfloat16)
    base = {
        "xb": xb,
        "wt": np.ascontiguousarray(W.T).astype(ml_dtypes.bfloat16),
        "bias": np.tile(np.asarray(b, np.float32)[None, :], (P, 1)),
        "iota": np.tile(np.arange(P, dtype=np.float32)[None, :], (P, 1)),
    }
    in_maps = []
    for k in range(plan.n_cores):
        m = dict(base)
        m["recip"] = plan.core_recip[k]
        for name, arr in plan.core_arrays[k].items():
            s = name.split("_")[1]
            if plan.Csum[s] == 0:
                continue
            m[name] = arr
        in_maps.append(m)
    return in_maps


def run(x, edge_index, n_nodes, W, b, trace=False, trace_cores=None):
    from concourse.bass_utils import run_bass_kernel_spmd

    x = np.asarray(x)
    edge_index = np.asarray(edge_index)
    W = np.asarray(W)
    b = np.asarray(b)
    n_nodes = int(n_nodes)
    row = edge_index[0].astype(np.int64)
    col = edge_index[1].astype(np.int64)

    plan = Plan(row, col, n_nodes)
    nc = build_program(plan, x.shape[1], W.shape[0])
    in_maps = make_in_maps(plan, x, W, b)
    res = run_bass_kernel_spmd(nc, in_maps, core_ids=list(range(plan.n_cores)),
                               trace=trace, trace_cores=trace_cores)
    out = np.concatenate(
        [res.results[k]["out"][:plan.d_core] for k in range(plan.n_cores)],
        axis=0)
    return np.ascontiguousarray(out, dtype=np.float32), res


def kernel(x, edge_index, n_nodes, W, b):
    out, _ = run(x, edge_index, n_nodes, W, b)
    return out


# revision 26
# speedup vs baseline: 1.1478x; 1.1478x over previous
"""GCN layer (gather + segment-sum + degree-normalize + linear) on 8 Trainium2 cores.

Strategy
--------
Destination-node sharding: core k owns dest rows [k*D, (k+1)*D), D = n_nodes/8.
The host groups each core's edges by 128-dest windows (dest-sorted); the
on-device segment-sum is done per 128-edge chunk with a PE matmul
(lhsT = gathered source features G [128 edge, 128 feat] bf16, rhs = selection
matrix S [128 edge, 128 dest] with S[e, j] = (col_rel[e] == j)), accumulating
aggT[feat, dest] in PSUM per window. S is built on DVE in 32-chunk batches
(one tensor_tensor is_equal of broadcast crel vs a constant iota row per
batch — batching amortizes the ~151-cycle DVE instruction overhead to
~140ns/chunk). 1/max(deg,1) is precomputed on the host (a pure function of
edge_index, like the gather indices) and applied per window as a
per-partition activation scale fused with the PSUM->SBUF copy on the Scalar
engine after the linear matmul. No PE transpose is needed: aggT in PSUM is
copied to SBUF (Scalar) and used directly as lhsT of the linear matmul
(out[j, f'] = sum_f aggT[f, j] * wt[f, f']). Bias rides as a replicated
[128, out_f] DVE add. No scatter-add, no collectives; each core writes its
own output slice and the host concatenates.

x is gathered in bf16 (256B/row). Measured on HW, dma_gather throughput is
descriptor-count-bound (~2.1-3 ns/desc sustained at 4 SWDGE queues,
regardless of 256B/512B payload or call size), so one 256B bf16 descriptor
per edge is the floor; bf16 keeps SDMA engine busy-time at half of the 512B
hi+lo split-f32 scheme's and gives rel err ~2.5e-3 (max-abs vs output
scale), well inside the 2e-2 gate. The gather stream is the critical path:
index tiles load first (the first call waits only on them), G blocks are
triple-buffered per stream so the serial Pool dispatcher never head-of-line
blocks on a consumer dependency, and DVE/PE/Scalar work (S-builds, window
accumulation, epilogues) all ride under the ~230us descriptor stream.

dma_gather facts (measured): idx arrays are int16, wrapped [16, N/16] and
replicated into all eight 16-partition groups; single_packet=False is required
for calls over 1024 indices. int16 limits a gather call's index range to
32768 rows, so edges are split into lo/hi source streams gathered from base
x[0] / x[32768].
"""
import sys
import os
sys.path.insert(0, "/opt/trn_rl_repo")

import numpy as np

P = 128
GATHER_SPLIT = 32768       # max rows addressable by a signed-int16 gather index
DEFAULT_BLK_CHUNKS = 32    # gather block size in 128-edge chunks
SBATCH = 32                # S-matrix build batch, in chunks (amortizes the
                           # ~151-cycle DVE instruction overhead)
N_CORES = 8


def _ceil_div(a, b):
    return -(-a // b)


def _wrap_idx(ix):
    """[N] int16 -> [128, N/16], idx i at [i%16, i//16], replicated into the
    eight 16-partition groups (the tx/rx Q7 cpus of every SWDGE queue each
    read their own group)."""
    n = len(ix)
    assert n % 16 == 0
    w = np.zeros((P, n // 16), np.int16)
    blk = ix.reshape(-1, 16).T
    for g in range(8):
        w[16 * g:16 * (g + 1), :] = blk
    return w


class Plan:
    """Host-side sharding: per-core per-stream edge arrays with a chunk
    structure (windows x chunk counts) identical across cores, so a single
    SPMD program serves all cores."""

    def __init__(self, row, col, n_nodes, n_cores=N_CORES,
                 blk_chunks=DEFAULT_BLK_CHUNKS, gather_split=GATHER_SPLIT):
        assert n_nodes % n_cores == 0
        self.n_cores = n_cores
        self.n_nodes = n_nodes
        self.d_core = n_nodes // n_cores
        self.n_win = _ceil_div(self.d_core, P)
        self.blk_chunks = blk_chunks
        self.gather_split = gather_split

        order = np.argsort(col, kind="stable")
        rs = row[order]
        cs = col[order]
        bounds = np.searchsorted(cs, np.arange(n_cores + 1) * self.d_core)

        # in-degree (clamped to 1) per node, laid out per core as
        # [P, n_win] f32 reciprocal: recip[j, w] = 1/deg of dest w*128+j
        deg = np.bincount(cs, minlength=n_nodes).astype(np.float32)
        deg = np.maximum(deg, 1.0)
        recip = (1.0 / deg)
        pad = self.n_win * P - self.d_core
        self.core_recip = []
        for k in range(n_cores):
            r = recip[k * self.d_core:(k + 1) * self.d_core]
            r = np.concatenate([r, np.zeros(pad, np.float32)])
            self.core_recip.append(
                np.ascontiguousarray(r.reshape(self.n_win, P).T))

        W = self.n_win
        per_core = []  # [k][stream] = (rows, local_cols, per-window counts)
        cnt = {"lo": np.zeros(W, np.int64), "hi": np.zeros(W, np.int64)}
        for k in range(n_cores):
            a, b = bounds[k], bounds[k + 1]
            r_k = rs[a:b]
            lc_k = cs[a:b] - k * self.d_core
            lo = r_k < gather_split
            streams = {}
            for sname, mask in (("lo", lo), ("hi", ~lo)):
                r_s = r_k[mask]
                lc_s = lc_k[mask]
                counts = np.bincount(lc_s >> 7, minlength=W)
                streams[sname] = (r_s, lc_s, counts)
                cnt[sname] = np.maximum(cnt[sname], -(-counts // P))
            per_core.append(streams)
        cnt["lo"] = np.maximum(cnt["lo"], 1)  # every window gets >=1 chunk
        self.cnt = cnt
        self.off = {s: np.concatenate([[0], np.cumsum(cnt[s])]) for s in cnt}
        self.Csum = {s: int(self.off[s][-1]) for s in cnt}
        self.NB = {s: _ceil_div(self.Csum[s], blk_chunks) for s in cnt}
        self.Npad = {s: self.NB[s] * blk_chunks * P for s in cnt}

        self.core_arrays = []
        for k in range(n_cores):
            arrs = {}
            for sname in ("lo", "hi"):
                r_s, lc_s, counts = per_core[k][sname]
                off = self.off[sname]
                base = 0 if sname == "lo" else gather_split
                gidx = np.zeros(self.Npad[sname], np.int16)
                crel = np.full(self.Csum[sname] * P, -1, np.int8)
                if len(r_s):
                    starts = np.concatenate([[0], np.cumsum(counts)])
                    adj = off[:-1] * P - starts[:-1]
                    dst = np.arange(len(r_s)) + adj[lc_s >> 7]
                    gidx[dst] = (r_s - base).astype(np.int16)
                    crel[dst] = (lc_s & 127).astype(np.int8)
                arrs[f"gidx_{sname}"] = _wrap_idx(gidx)
                # [P edge-slot, Csum chunk] f32 (tensor_scalar is_equal
                # requires a float32 scalar operand)
                arrs[f"crel_{sname}"] = np.ascontiguousarray(
                    crel.reshape(self.Csum[sname], P).T.astype(np.float32))
            self.core_arrays.append(arrs)

    @property
    def total_chunks(self):
        return self.Csum["lo"] + self.Csum["hi"]


def _patch_swdge_lane_by_queue():
    """Pin each dma_gather's DMASW semaphore lane to its SWDGE queue number.

    Tile assigns DMASW lanes round-robin in scheduled order, which breaks when
    instructions on different queues (whose completions are only FIFO within a
    queue) share a lane. Two lanes per queue keep per-lane completion in-order
    and let a call's desc-gen overlap the previous same-queue call's DMA
    completion.
    """
    import concourse.tile_sem_assignment as tsa
    from concourse import mybir
    if getattr(tsa.TileClockTick, "_lane_by_queue_patch", False):
        return
    orig = tsa.TileClockTick._assign_tick

    def patched(self, inst):
        if isinstance(inst, mybir.InstDMAGatherAnt):
            if not hasattr(self, "_q_lane_ctr"):
                self._q_lane_ctr = {}
            q = inst.queue_num
            n = self._q_lane_ctr.get(q, 0)
            self._q_lane_ctr[q] = n + 1
            saved = self.next_sw_dma_idx
            self.next_sw_dma_idx = q * 2 + (n % 2)
            try:
                return orig(self, inst)
            finally:
                self.next_sw_dma_idx = saved
        return orig(self, inst)

    tsa.TileClockTick._assign_tick = patched
    tsa.TileClockTick._lane_by_queue_patch = True


def build_program(plan, in_f, out_f):
    """Emit the SPMD Bass program (shared by all cores)."""
    from concourse import bacc, mybir
    import concourse.tile as tile
    from contextlib import ExitStack

    _patch_swdge_lane_by_queue()

    f32 = mybir.dt.float32
    i16 = mybir.dt.int16
    bf16 = mybir.dt.bfloat16

    W = plan.n_win
    BLK = plan.blk_chunks

    nc = bacc.Bacc("TRN2", target_bir_lowering=False, debug=False,
                   num_devices=plan.n_cores, num_swdge_queues=4)

    x_d = nc.dram_tensor("xb", [plan.n_nodes, in_f], bf16,
                         kind="ExternalInput")
    wt_d = nc.dram_tensor("wt", [in_f, out_f], bf16, kind="ExternalInput")
    bias_d = nc.dram_tensor("bias", [P, out_f], f32, kind="ExternalInput")
    iota_d = nc.dram_tensor("iota", [P, P], f32, kind="ExternalInput")
    recip_d = nc.dram_tensor("recip", [P, W], f32, kind="ExternalInput")
    gidx_d, crel_d = {}, {}
    for s in ("lo", "hi"):
        if plan.Csum[s] == 0:
            continue
        gidx_d[s] = nc.dram_tensor(f"gidx_{s}", [P, plan.Npad[s] // 16], i16,
                                   kind="ExternalInput")
        crel_d[s] = nc.dram_tensor(f"crel_{s}", [P, plan.Csum[s]], f32,
                                   kind="ExternalInput")
    out_d = nc.dram_tensor("out", [W * P, out_f], f32, kind="ExternalOutput")

    x_base = {"lo": x_d[:], "hi": x_d[plan.gather_split:, :]}

    with tile.TileContext(nc) as tc, ExitStack() as ctx:
        cpool = ctx.enter_context(tc.tile_pool(name="const", bufs=1))
        gpool = {s: ctx.enter_context(tc.tile_pool(name=f"g_{s}", bufs=3))
                 for s in ("lo", "hi")}
        spool = {s: ctx.enter_context(tc.tile_pool(name=f"s_{s}", bufs=3))
                 for s in ("lo", "hi")}
        epool = ctx.enter_context(tc.tile_pool(name="epi", bufs=3))
        apool = ctx.enter_context(tc.tile_pool(name="psum_a", bufs=4,
                                               space="PSUM"))
        hpool = ctx.enter_context(tc.tile_pool(name="psum_h", bufs=2,
                                               space="PSUM"))

        # ---- warm-up gathers ----
        # The first dma_gather on each cold SWDGE queue takes ~16us before
        # its first packet lands (ucode cold start). One tiny gather per
        # queue, each with its OWN output tile (a shared tile would chain
        # them through WAW deps), pays that cost on all four queues
        # concurrently while the index tiles load.
        warm_idx = cpool.tile([P, 8], i16, name="warmidx")
        nc.vector.memset(warm_idx[:], 0)
        warm_outs = []
        for q in range(4):
            wo = cpool.tile([P, in_f], bf16, name=f"warmout{q}")
            nc.gpsimd.dma_gather(
                wo[:].rearrange("p (c e) -> p c e", e=in_f),
                x_d[:],
                warm_idx[:],
                P, P, in_f,
                single_packet=False,
                queue_num=q,
            )
            warm_outs.append(wo)
        # touch each warm-up tile so the gathers aren't dead-code'd
        for q, wo in enumerate(warm_outs):
            wa = cpool.tile([P, 1], f32, name=f"warmacc{q}")
            nc.vector.tensor_tensor(out=wa[:], in0=wo[:, :2].bitcast(f32),
                                    in1=wo[:, :2].bitcast(f32),
                                    op=mybir.AluOpType.mult)

        # ---- constants ----
        # gidx first: the first real gather call (the critical-path wall)
        # waits only on its index tile, not the whole constant load train.
        # The head (first NHEAD blocks) is a SEPARATE tile: Tile tracks
        # dependencies at tile granularity, so a single sliced tensor would
        # make the first gather wait for the full ~1.2MB load (~35us on the
        # ~50 GB/s sync HWDGE path) instead of the ~130KB head.
        NHEAD = 2                    # blocks served from the head tile
        HEADB = NHEAD * BLK * P // 16  # head size in wrapped idx columns
        gidx_head, gidx_tail, crel_t = {}, {}, {}
        for s in ("lo", "hi"):
            if plan.Csum[s] == 0:
                continue
            ncols = plan.Npad[s] // 16
            cut = min(HEADB, ncols)
            gh = cpool.tile([P, cut], i16, name=f"gidxh{s}")
            nc.sync.dma_start(out=gh[:], in_=gidx_d[s][:, :cut])
            gidx_head[s] = gh
        for s in ("lo", "hi"):
            if plan.Csum[s] == 0:
                continue
            ncols = plan.Npad[s] // 16
            cut = min(HEADB, ncols)
            if cut < ncols:
                gt_ = cpool.tile([P, ncols - cut], i16, name=f"gidxt{s}")
                nc.sync.dma_start(out=gt_[:], in_=gidx_d[s][:, cut:])
                gidx_tail[s] = gt_
        for s in ("lo", "hi"):
            if plan.Csum[s] == 0:
                continue
            cri = cpool.tile([P, plan.Csum[s]], f32, name=f"crel{s}")
            nc.sync.dma_start(out=cri[:], in_=crel_d[s][:])
            crel_t[s] = cri
        iota_t = cpool.tile([P, P], f32)
        nc.sync.dma_start(out=iota_t[:], in_=iota_d[:])
        wt_t = cpool.tile([in_f, out_f], bf16)
        nc.sync.dma_start(out=wt_t[:], in_=wt_d[:])
        bias_t = cpool.tile([P, out_f], f32)
        nc.sync.dma_start(out=bias_t[:], in_=bias_d[:])
        recip_t = cpool.tile([P, W], f32)
        nc.sync.dma_start(out=recip_t[:], in_=recip_d[:])

        # ---- lazily-emitted gather blocks ----
        # Queue assignment is static per (stream, block parity) so each pool
        # tag's DMA semaphore lane stays on one SWDGE queue.
        g_tiles = {}
        qctr = [0]
        HB = BLK // 2  # half-block chunks; one gather call per half, own queue

        def get_g(s, b):
            if (s, b) not in g_tiles:
                gt = gpool[s].tile([P, BLK * in_f], bf16, name=f"G{s}{b}",
                                   tag=f"G{s}{b % 2}")
                for h in range(2):
                    c0 = b * BLK + h * HB
                    nch = min(HB, max(plan.Csum[s] - c0, 0))
                    if nch == 0:
                        continue
                    a = c0 * P // 16
                    z = (c0 + nch) * P // 16
                    if b < NHEAD:
                        idx_ap = gidx_head[s][:, a:z]
                    else:
                        idx_ap = gidx_tail[s][:, a - HEADB:z - HEADB]
                    nc.gpsimd.dma_gather(
                        gt[:, h * HB * in_f:(h * HB + nch) * in_f]
                        .rearrange("p (c e) -> p c e", e=in_f),
                        x_base[s],
                        idx_ap,
                        nch * P,
                        nch * P,
                        in_f,
                        single_packet=False,
                        queue_num=qctr[0] % 4,
                    )
                    qctr[0] += 1
                g_tiles[(s, b)] = gt
            return g_tiles[(s, b)]

        # ---- lazily-emitted batched S builds (one DVE tensor_tensor
        # is_equal per SBATCH chunks; the ~151-cycle instruction overhead
        # amortizes to ~138ns/chunk) ----
        s_tiles = {}

        def get_s(s, sb):
            if (s, sb) not in s_tiles:
                st = spool[s].tile([P, SBATCH * P], bf16, name=f"S{s}{sb}",
                                   tag=f"S{s}")
                nb = min(SBATCH, plan.Csum[s] - sb * SBATCH)
                in0 = crel_t[s][:, sb * SBATCH:sb * SBATCH + nb] \
                    .to_broadcast([P, nb, P])
                in1 = iota_t[:][:, None, :].to_broadcast([P, nb, P])
                outv = st[:].rearrange("p (b j) -> p b j", j=P)[:, :nb, :]
                nc.vector.tensor_tensor(out=outv, in0=in0, in1=in1,
                                        op=mybir.AluOpType.is_equal)
                s_tiles[(s, sb)] = st
            return s_tiles[(s, sb)]

        # ---- pre-issue every gather call, interleaved by stream progress,
        # so the Pool engine always has ready calls on all 4 queues ----
        order = sorted(
            [(s, b) for s in ("lo", "hi") for b in range(plan.NB[s])],
            key=lambda sb: (sb[1] + 0.5) / plan.NB[sb[0]])
        for s, b in order:
            get_g(s, b)

        # ---- main window loop ----
        for w in range(W):
            chunks = []
            for s in ("lo", "hi"):
                chunks += [(s, c) for c in
                           range(plan.off[s][w], plan.off[s][w + 1])]
            psum_aggT = apool.tile([P, in_f], f32, tag="aggT",
                                   name=f"aggT{w}")
            n = len(chunks)
            for i, (s, c) in enumerate(chunks):
                b, slot = divmod(c, BLK)
                sb, ssub = divmod(c, SBATCH)
                gt = get_g(s, b)
                st = get_s(s, sb)
                nc.tensor.matmul(
                    out=psum_aggT[:],
                    lhsT=gt[:, slot * in_f:(slot + 1) * in_f],
                    rhs=st[:, ssub * P:(ssub + 1) * P],
                    start=(i == 0), stop=(i == n - 1))

            hT_t = epool.tile([P, P], bf16, tag="hT", name=f"hT{w}")
            nc.scalar.activation(out=hT_t[:], in_=psum_aggT[:],
                                 func=mybir.ActivationFunctionType.Copy)
            out_p = hpool.tile([P, out_f], f32, tag="outp", name=f"outp{w}")
            nc.tensor.matmul(out=out_p[:], lhsT=hT_t[:], rhs=wt_t[:],
                             start=True, stop=True)
            out_s = epool.tile([P, out_f], f32, tag="outs", name=f"outs{w}")
            nc.scalar.activation(out=out_s[:], in_=out_p[:],
                                 func=mybir.ActivationFunctionType.Copy,
                                 scale=recip_t[:, w:w + 1])
            out_t = epool.tile([P, out_f], f32, tag="outt", name=f"outt{w}")
            nc.vector.tensor_tensor(out=out_t[:], in0=out_s[:], in1=bias_t[:],
                                    op=mybir.AluOpType.add)
            nc.sync.dma_start(out=out_d[w * P:(w + 1) * P, :], in_=out_t[:])

    nc.compile()
    return nc


def make_in_maps(plan, x, W, b):
    in_f = x.shape[1]
    out_f = W.shape[0]
    import ml_dtypes
    xb = np.ascontiguousarray(x, dtype=np.float32).astype(ml_dtypes.bfloat16)
    base = {
        "xb": xb,
        "wt": np.ascontiguousarray(W.T).astype(ml_dtypes.bfloat16),
        "bias": np.tile(np.asarray(b, np.float32)[None, :], (P, 1)),
        "iota": np.tile(np.arange(P, dtype=np.float32)[None, :], (P, 1)),
    }
    in_maps = []
    for k in range(plan.n_cores):
        m = dict(base)
        m["recip"] = plan.core_recip[k]
        for name, arr in plan.core_arrays[k].items():
            s = name.split("_")[1]
            if plan.Csum[s] == 0:
                continue
            m[name] = arr
        in_maps.append(m)
    return in_maps


def run(x, edge_index, n_nodes, W, b, trace=False, trace_cores=None):
    from concourse.bass_utils import run_bass_kernel_spmd

    x = np.asarray(x)
    edge_index = np.asarray(edge_index)
    W = np.asarray(W)
    b = np.asarray(b)
    n_nodes = int(n_nodes)
    row = edge_index[0].astype(np.int64)
    col = edge_index[1].astype(np.int64)

    plan = Plan(row, col, n_nodes)
    nc = build_program(plan, x.shape[1], W.shape[0])
    in_maps = make_in_maps(plan, x, W, b)
    res = run_bass_kernel_spmd(nc, in_maps, core_ids=list(range(plan.n_cores)),
                               trace=trace, trace_cores=trace_cores)
    out = np.concatenate(
        [res.results[k]["out"][:plan.d_core] for k in range(plan.n_cores)],
        axis=0)
    return np.ascontiguousarray(out, dtype=np.float32), res


def kernel(x, edge_index, n_nodes, W, b):
    out, _ = run(x, edge_index, n_nodes, W, b)
    return out


# revision 27
# speedup vs baseline: 1.1954x; 1.0415x over previous
"""GCN layer (gather + segment-sum + degree-normalize + linear) on 8 Trainium2 cores.

Strategy
--------
Destination-node sharding: core k owns dest rows [k*D, (k+1)*D), D = n_nodes/8.
The host groups each core's edges by 128-dest windows (dest-sorted); the
on-device segment-sum is done per 128-edge chunk with a PE matmul
(lhsT = gathered source features G [128 edge, 128 feat] bf16, rhs = selection
matrix S [128 edge, 128 dest] with S[e, j] = (col_rel[e] == j)), accumulating
aggT[feat, dest] in PSUM per window. S is built on DVE in 32-chunk batches
(one tensor_tensor is_equal of broadcast crel vs a constant iota row per
batch — batching amortizes the ~151-cycle DVE instruction overhead to
~140ns/chunk). 1/max(deg,1) is precomputed on the host (a pure function of
edge_index, like the gather indices) and applied per window as a
per-partition activation scale fused with the PSUM->SBUF copy on the Scalar
engine after the linear matmul. No PE transpose is needed: aggT in PSUM is
copied to SBUF (Scalar) and used directly as lhsT of the linear matmul
(out[j, f'] = sum_f aggT[f, j] * wt[f, f']). Bias rides as a replicated
[128, out_f] DVE add. No scatter-add, no collectives; each core writes its
own output slice and the host concatenates.

x is gathered in bf16 (256B/row). Measured on HW, dma_gather throughput is
descriptor-count-bound (~2.1-3 ns/desc sustained at 4 SWDGE queues,
regardless of 256B/512B payload or call size), so one 256B bf16 descriptor
per edge is the floor; bf16 keeps SDMA engine busy-time at half of the 512B
hi+lo split-f32 scheme's and gives rel err ~2.5e-3 (max-abs vs output
scale), well inside the 2e-2 gate. The gather stream is the critical path:
index tiles load first (the first call waits only on them), G blocks are
triple-buffered per stream so the serial Pool dispatcher never head-of-line
blocks on a consumer dependency, and DVE/PE/Scalar work (S-builds, window
accumulation, epilogues) all ride under the ~230us descriptor stream.

dma_gather facts (measured): idx arrays are int16, wrapped [16, N/16] and
replicated into all eight 16-partition groups; single_packet=False is required
for calls over 1024 indices. int16 limits a gather call's index range to
32768 rows, so edges are split into lo/hi source streams gathered from base
x[0] / x[32768].
"""
import sys
import os
sys.path.insert(0, "/opt/trn_rl_repo")

import numpy as np

P = 128
GATHER_SPLIT = 32768       # max rows addressable by a signed-int16 gather index
DEFAULT_BLK_CHUNKS = 32    # gather block size in 128-edge chunks
SBATCH = 32                # S-matrix build batch, in chunks (amortizes the
                           # ~151-cycle DVE instruction overhead)
N_CORES = 8


def _ceil_div(a, b):
    return -(-a // b)


def _wrap_idx(ix):
    """[N] int16 -> [128, N/16], idx i at [i%16, i//16], replicated into the
    eight 16-partition groups (the tx/rx Q7 cpus of every SWDGE queue each
    read their own group)."""
    n = len(ix)
    assert n % 16 == 0
    w = np.zeros((P, n // 16), np.int16)
    blk = ix.reshape(-1, 16).T
    for g in range(8):
        w[16 * g:16 * (g + 1), :] = blk
    return w


class Plan:
    """Host-side sharding: per-core per-stream edge arrays with a chunk
    structure (windows x chunk counts) identical across cores, so a single
    SPMD program serves all cores."""

    def __init__(self, row, col, n_nodes, n_cores=N_CORES,
                 blk_chunks=DEFAULT_BLK_CHUNKS, gather_split=GATHER_SPLIT):
        assert n_nodes % n_cores == 0
        self.n_cores = n_cores
        self.n_nodes = n_nodes
        self.d_core = n_nodes // n_cores
        self.n_win = _ceil_div(self.d_core, P)
        self.blk_chunks = blk_chunks
        self.gather_split = gather_split

        order = np.argsort(col, kind="stable")
        rs = row[order]
        cs = col[order]
        bounds = np.searchsorted(cs, np.arange(n_cores + 1) * self.d_core)

        # in-degree (clamped to 1) per node, laid out per core as
        # [P, n_win] f32 reciprocal: recip[j, w] = 1/deg of dest w*128+j
        deg = np.bincount(cs, minlength=n_nodes).astype(np.float32)
        deg = np.maximum(deg, 1.0)
        recip = (1.0 / deg)
        pad = self.n_win * P - self.d_core
        self.core_recip = []
        for k in range(n_cores):
            r = recip[k * self.d_core:(k + 1) * self.d_core]
            r = np.concatenate([r, np.zeros(pad, np.float32)])
            self.core_recip.append(
                np.ascontiguousarray(r.reshape(self.n_win, P).T))

        W = self.n_win
        per_core = []  # [k][stream] = (rows, local_cols, per-window counts)
        cnt = {"lo": np.zeros(W, np.int64), "hi": np.zeros(W, np.int64)}
        for k in range(n_cores):
            a, b = bounds[k], bounds[k + 1]
            r_k = rs[a:b]
            lc_k = cs[a:b] - k * self.d_core
            lo = r_k < gather_split
            streams = {}
            for sname, mask in (("lo", lo), ("hi", ~lo)):
                r_s = r_k[mask]
                lc_s = lc_k[mask]
                counts = np.bincount(lc_s >> 7, minlength=W)
                streams[sname] = (r_s, lc_s, counts)
                cnt[sname] = np.maximum(cnt[sname], -(-counts // P))
            per_core.append(streams)
        cnt["lo"] = np.maximum(cnt["lo"], 1)  # every window gets >=1 chunk
        self.cnt = cnt
        self.off = {s: np.concatenate([[0], np.cumsum(cnt[s])]) for s in cnt}
        self.Csum = {s: int(self.off[s][-1]) for s in cnt}
        self.NB = {s: _ceil_div(self.Csum[s], blk_chunks) for s in cnt}
        self.Npad = {s: self.NB[s] * blk_chunks * P for s in cnt}

        self.core_arrays = []
        for k in range(n_cores):
            arrs = {}
            for sname in ("lo", "hi"):
                r_s, lc_s, counts = per_core[k][sname]
                off = self.off[sname]
                base = 0 if sname == "lo" else gather_split
                gidx = np.zeros(self.Npad[sname], np.int16)
                crel = np.full(self.Csum[sname] * P, -1, np.int8)
                if len(r_s):
                    starts = np.concatenate([[0], np.cumsum(counts)])
                    adj = off[:-1] * P - starts[:-1]
                    dst = np.arange(len(r_s)) + adj[lc_s >> 7]
                    gidx[dst] = (r_s - base).astype(np.int16)
                    crel[dst] = (lc_s & 127).astype(np.int8)
                arrs[f"gidx_{sname}"] = _wrap_idx(gidx)
                # [P edge-slot, Csum chunk] f32 (tensor_scalar is_equal
                # requires a float32 scalar operand)
                arrs[f"crel_{sname}"] = np.ascontiguousarray(
                    crel.reshape(self.Csum[sname], P).T.astype(np.float32))
            self.core_arrays.append(arrs)

    @property
    def total_chunks(self):
        return self.Csum["lo"] + self.Csum["hi"]


def _patch_swdge_lane_by_queue():
    """Pin each dma_gather's DMASW semaphore lane to its SWDGE queue number.

    Tile assigns DMASW lanes round-robin in scheduled order, which breaks when
    instructions on different queues (whose completions are only FIFO within a
    queue) share a lane. Two lanes per queue keep per-lane completion in-order
    and let a call's desc-gen overlap the previous same-queue call's DMA
    completion.
    """
    import concourse.tile_sem_assignment as tsa
    from concourse import mybir
    if getattr(tsa.TileClockTick, "_lane_by_queue_patch", False):
        return
    orig = tsa.TileClockTick._assign_tick

    def patched(self, inst):
        if isinstance(inst, mybir.InstDMAGatherAnt):
            if not hasattr(self, "_q_lane_ctr"):
                self._q_lane_ctr = {}
            q = inst.queue_num
            n = self._q_lane_ctr.get(q, 0)
            self._q_lane_ctr[q] = n + 1
            saved = self.next_sw_dma_idx
            self.next_sw_dma_idx = q * 2 + (n % 2)
            try:
                return orig(self, inst)
            finally:
                self.next_sw_dma_idx = saved
        return orig(self, inst)

    tsa.TileClockTick._assign_tick = patched
    tsa.TileClockTick._lane_by_queue_patch = True


def build_program(plan, in_f, out_f):
    """Emit the SPMD Bass program (shared by all cores)."""
    from concourse import bacc, mybir
    import concourse.tile as tile
    from contextlib import ExitStack

    _patch_swdge_lane_by_queue()

    f32 = mybir.dt.float32
    i16 = mybir.dt.int16
    bf16 = mybir.dt.bfloat16

    W = plan.n_win
    BLK = plan.blk_chunks

    nc = bacc.Bacc("TRN2", target_bir_lowering=False, debug=False,
                   num_devices=plan.n_cores, num_swdge_queues=4)

    x_d = nc.dram_tensor("xb", [plan.n_nodes, in_f], bf16,
                         kind="ExternalInput")
    wt_d = nc.dram_tensor("wt", [in_f, out_f], bf16, kind="ExternalInput")
    bias_d = nc.dram_tensor("bias", [P, out_f], f32, kind="ExternalInput")
    iota_d = nc.dram_tensor("iota", [P, P], f32, kind="ExternalInput")
    recip_d = nc.dram_tensor("recip", [P, W], f32, kind="ExternalInput")
    gidx_d, crel_d = {}, {}
    for s in ("lo", "hi"):
        if plan.Csum[s] == 0:
            continue
        gidx_d[s] = nc.dram_tensor(f"gidx_{s}", [P, plan.Npad[s] // 16], i16,
                                   kind="ExternalInput")
        crel_d[s] = nc.dram_tensor(f"crel_{s}", [P, plan.Csum[s]], f32,
                                   kind="ExternalInput")
    out_d = nc.dram_tensor("out", [W * P, out_f], f32, kind="ExternalOutput")

    x_base = {"lo": x_d[:], "hi": x_d[plan.gather_split:, :]}

    with tile.TileContext(nc) as tc, ExitStack() as ctx:
        cpool = ctx.enter_context(tc.tile_pool(name="const", bufs=1))
        gpool = {s: ctx.enter_context(tc.tile_pool(name=f"g_{s}", bufs=4))
                 for s in ("lo", "hi")}
        spool = {s: ctx.enter_context(tc.tile_pool(name=f"s_{s}", bufs=3))
                 for s in ("lo", "hi")}
        epool = ctx.enter_context(tc.tile_pool(name="epi", bufs=3))
        apool = ctx.enter_context(tc.tile_pool(name="psum_a", bufs=4,
                                               space="PSUM"))
        hpool = ctx.enter_context(tc.tile_pool(name="psum_h", bufs=2,
                                               space="PSUM"))

        # ---- warm-up gathers ----
        # The first dma_gather on each cold SWDGE queue takes ~16us before
        # its first packet lands (ucode cold start). One tiny gather per
        # queue, each with its OWN output tile (a shared tile would chain
        # them through WAW deps), pays that cost on all four queues
        # concurrently while the index tiles load.
        warm_idx = cpool.tile([P, 8], i16, name="warmidx")
        nc.vector.memset(warm_idx[:], 0)
        warm_outs = []
        for q in range(4):
            wo = cpool.tile([P, in_f], bf16, name=f"warmout{q}")
            nc.gpsimd.dma_gather(
                wo[:].rearrange("p (c e) -> p c e", e=in_f),
                x_d[:],
                warm_idx[:],
                P, P, in_f,
                single_packet=False,
                queue_num=q,
            )
            warm_outs.append(wo)
        # touch each warm-up tile so the gathers aren't dead-code'd
        for q, wo in enumerate(warm_outs):
            wa = cpool.tile([P, 1], f32, name=f"warmacc{q}")
            nc.vector.tensor_tensor(out=wa[:], in0=wo[:, :2].bitcast(f32),
                                    in1=wo[:, :2].bitcast(f32),
                                    op=mybir.AluOpType.mult)

        # ---- constants ----
        # gidx first: the first real gather call (the critical-path wall)
        # waits only on its index tile, not the whole constant load train.
        # The head (first NHEAD blocks) is a SEPARATE tile: Tile tracks
        # dependencies at tile granularity, so a single sliced tensor would
        # make the first gather wait for the full ~1.2MB load (~35us on the
        # ~50 GB/s sync HWDGE path) instead of the ~130KB head.
        NHEAD = 2                    # blocks served from the head tile
        HEADB = NHEAD * BLK * P // 16  # head size in wrapped idx columns
        gidx_head, gidx_tail, crel_t = {}, {}, {}
        for s in ("lo", "hi"):
            if plan.Csum[s] == 0:
                continue
            ncols = plan.Npad[s] // 16
            cut = min(HEADB, ncols)
            gh = cpool.tile([P, cut], i16, name=f"gidxh{s}")
            nc.sync.dma_start(out=gh[:], in_=gidx_d[s][:, :cut])
            gidx_head[s] = gh
        for s in ("lo", "hi"):
            if plan.Csum[s] == 0:
                continue
            ncols = plan.Npad[s] // 16
            cut = min(HEADB, ncols)
            if cut < ncols:
                gt_ = cpool.tile([P, ncols - cut], i16, name=f"gidxt{s}")
                nc.sync.dma_start(out=gt_[:], in_=gidx_d[s][:, cut:])
                gidx_tail[s] = gt_
        for s in ("lo", "hi"):
            if plan.Csum[s] == 0:
                continue
            cri = cpool.tile([P, plan.Csum[s]], f32, name=f"crel{s}")
            nc.sync.dma_start(out=cri[:], in_=crel_d[s][:])
            crel_t[s] = cri
        iota_t = cpool.tile([P, P], f32)
        nc.sync.dma_start(out=iota_t[:], in_=iota_d[:])
        wt_t = cpool.tile([in_f, out_f], bf16)
        nc.sync.dma_start(out=wt_t[:], in_=wt_d[:])
        bias_t = cpool.tile([P, out_f], f32)
        nc.sync.dma_start(out=bias_t[:], in_=bias_d[:])
        recip_t = cpool.tile([P, W], f32)
        nc.sync.dma_start(out=recip_t[:], in_=recip_d[:])

        # ---- lazily-emitted gather blocks ----
        # Queue assignment is static per (stream, block parity) so each pool
        # tag's DMA semaphore lane stays on one SWDGE queue.
        g_tiles = {}
        qctr = [0]
        HB = BLK // 2  # half-block chunks; one gather call per half, own queue

        def get_g(s, b):
            if (s, b) not in g_tiles:
                gt = gpool[s].tile([P, BLK * in_f], bf16, name=f"G{s}{b}",
                                   tag=f"G{s}{b % 2}")
                for h in range(2):
                    c0 = b * BLK + h * HB
                    nch = min(HB, max(plan.Csum[s] - c0, 0))
                    if nch == 0:
                        continue
                    a = c0 * P // 16
                    z = (c0 + nch) * P // 16
                    if b < NHEAD:
                        idx_ap = gidx_head[s][:, a:z]
                    else:
                        idx_ap = gidx_tail[s][:, a - HEADB:z - HEADB]
                    nc.gpsimd.dma_gather(
                        gt[:, h * HB * in_f:(h * HB + nch) * in_f]
                        .rearrange("p (c e) -> p c e", e=in_f),
                        x_base[s],
                        idx_ap,
                        nch * P,
                        nch * P,
                        in_f,
                        single_packet=False,
                        queue_num=qctr[0] % 4,
                    )
                    qctr[0] += 1
                g_tiles[(s, b)] = gt
            return g_tiles[(s, b)]

        # ---- lazily-emitted batched S builds (one DVE tensor_tensor
        # is_equal per SBATCH chunks; the ~151-cycle instruction overhead
        # amortizes to ~138ns/chunk) ----
        s_tiles = {}

        def get_s(s, sb):
            if (s, sb) not in s_tiles:
                st = spool[s].tile([P, SBATCH * P], bf16, name=f"S{s}{sb}",
                                   tag=f"S{s}")
                nb = min(SBATCH, plan.Csum[s] - sb * SBATCH)
                in0 = crel_t[s][:, sb * SBATCH:sb * SBATCH + nb] \
                    .to_broadcast([P, nb, P])
                in1 = iota_t[:][:, None, :].to_broadcast([P, nb, P])
                outv = st[:].rearrange("p (b j) -> p b j", j=P)[:, :nb, :]
                nc.vector.tensor_tensor(out=outv, in0=in0, in1=in1,
                                        op=mybir.AluOpType.is_equal)
                s_tiles[(s, sb)] = st
            return s_tiles[(s, sb)]

        # ---- pre-issue every gather call, interleaved by stream progress,
        # so the Pool engine always has ready calls on all 4 queues ----
        order = sorted(
            [(s, b) for s in ("lo", "hi") for b in range(plan.NB[s])],
            key=lambda sb: (sb[1] + 0.5) / plan.NB[sb[0]])
        for s, b in order:
            get_g(s, b)

        # ---- main window loop ----
        for w in range(W):
            chunks = []
            for s in ("lo", "hi"):
                chunks += [(s, c) for c in
                           range(plan.off[s][w], plan.off[s][w + 1])]
            psum_aggT = apool.tile([P, in_f], f32, tag="aggT",
                                   name=f"aggT{w}")
            n = len(chunks)
            for i, (s, c) in enumerate(chunks):
                b, slot = divmod(c, BLK)
                sb, ssub = divmod(c, SBATCH)
                gt = get_g(s, b)
                st = get_s(s, sb)
                nc.tensor.matmul(
                    out=psum_aggT[:],
                    lhsT=gt[:, slot * in_f:(slot + 1) * in_f],
                    rhs=st[:, ssub * P:(ssub + 1) * P],
                    start=(i == 0), stop=(i == n - 1))

            hT_t = epool.tile([P, P], bf16, tag="hT", name=f"hT{w}")
            nc.scalar.activation(out=hT_t[:], in_=psum_aggT[:],
                                 func=mybir.ActivationFunctionType.Copy)
            out_p = hpool.tile([P, out_f], f32, tag="outp", name=f"outp{w}")
            nc.tensor.matmul(out=out_p[:], lhsT=hT_t[:], rhs=wt_t[:],
                             start=True, stop=True)
            out_s = epool.tile([P, out_f], f32, tag="outs", name=f"outs{w}")
            nc.scalar.activation(out=out_s[:], in_=out_p[:],
                                 func=mybir.ActivationFunctionType.Copy,
                                 scale=recip_t[:, w:w + 1])
            out_t = epool.tile([P, out_f], f32, tag="outt", name=f"outt{w}")
            nc.vector.tensor_tensor(out=out_t[:], in0=out_s[:], in1=bias_t[:],
                                    op=mybir.AluOpType.add)
            nc.sync.dma_start(out=out_d[w * P:(w + 1) * P, :], in_=out_t[:])

    nc.compile()
    return nc


def make_in_maps(plan, x, W, b):
    in_f = x.shape[1]
    out_f = W.shape[0]
    import ml_dtypes
    xb = np.ascontiguousarray(x, dtype=np.float32).astype(ml_dtypes.bfloat16)
    base = {
        "xb": xb,
        "wt": np.ascontiguousarray(W.T).astype(ml_dtypes.bfloat16),
        "bias": np.tile(np.asarray(b, np.float32)[None, :], (P, 1)),
        "iota": np.tile(np.arange(P, dtype=np.float32)[None, :], (P, 1)),
    }
    in_maps = []
    for k in range(plan.n_cores):
        m = dict(base)
        m["recip"] = plan.core_recip[k]
        for name, arr in plan.core_arrays[k].items():
            s = name.split("_")[1]
            if plan.Csum[s] == 0:
                continue
            m[name] = arr
        in_maps.append(m)
    return in_maps


def run(x, edge_index, n_nodes, W, b, trace=False, trace_cores=None):
    from concourse.bass_utils import run_bass_kernel_spmd

    x = np.asarray(x)
    edge_index = np.asarray(edge_index)
    W = np.asarray(W)
    b = np.asarray(b)
    n_nodes = int(n_nodes)
    row = edge_index[0].astype(np.int64)
    col = edge_index[1].astype(np.int64)

    plan = Plan(row, col, n_nodes)
    nc = build_program(plan, x.shape[1], W.shape[0])
    in_maps = make_in_maps(plan, x, W, b)
    res = run_bass_kernel_spmd(nc, in_maps, core_ids=list(range(plan.n_cores)),
                               trace=trace, trace_cores=trace_cores)
    out = np.concatenate(
        [res.results[k]["out"][:plan.d_core] for k in range(plan.n_cores)],
        axis=0)
    return np.ascontiguousarray(out, dtype=np.float32), res


def kernel(x, edge_index, n_nodes, W, b):
    out, _ = run(x, edge_index, n_nodes, W, b)
    return out
